# revision 1
# baseline (speedup 1.0000x reference)
"""Trainium2 Bass kernel for nn_Attention_63513976373985.

Strategy: pure data-parallel over the batch dim B=64 across 8 NeuronCores
(8 batches per core, all params replicated, no collectives). Inside each
core, per-batch pipeline:
  X = d2[b]            [S=512, F=512]  fp16 in DRAM, loaded TRANSPOSED via
                       the xbar DMA-transpose (2-byte dtypes only)
  d3T = relu(w1.T @ XT + b1)           [C, S]   (fp16 matmul, f32 PSUM)
  tv  = tanh(XT.T @ wv)                [S, C]   natural layout for vs
  per head h: zsT = Wtop[h].T @ d3T  (+ zconst[h,b] per-partition ACT bias,
              where zconst = relu(d1@w1+b1) @ Wbot[h] — the d4 half of d5)
              usT = tanh(zsT + zconst)           [C, S]
  atts = blockdiag(P) matvec over usT            [H, S]
  softmax over S (ACT exp w/ accum_out, DVE reciprocal; 1/Z folded into
  the vs eviction as a per-partition scale)
  scoresT via PE transpose, vs = scoresT.T @ tv  [H, C]
  V slabs via PE transpose, out = relu(V.T @ wcc + bcc)  [BLOC, 128]

Everything runs fp16 (same 10-bit mantissa as tf32; fp32 PSUM
accumulate). No float32r anywhere: f32r DMA loads engage a rounding mode
that corrupts concurrently-executing fp16 xbar DMA-transposes on this HW.
"""
import sys

if "/opt/trn_rl_repo" not in sys.path:
    sys.path.insert(0, "/opt/trn_rl_repo")

import numpy as np

H, F, C, S, B = 8, 512, 256, 512, 64
NCORES = 8
BLOC = B // NCORES  # 8
OUTF = 128

_CACHE = {}


def build_nc():
    import concourse.bass as bass  # noqa: F401
    import concourse.mybir as mybir
    import concourse.tile as tile
    from concourse import bacc
    from contextlib import ExitStack

    f32 = mybir.dt.float32
    f16 = mybir.dt.float16
    AF = mybir.ActivationFunctionType

    nc = bacc.Bacc("TRN2", target_bir_lowering=False, debug=False,
                   num_devices=NCORES)

    # ---- DRAM parameters (per-core shard shapes) ----
    d2_d = nc.dram_tensor("d2", [BLOC, S, F], f16, kind="ExternalInput")
    d1t_d = nc.dram_tensor("d1t", [128, 4, BLOC], f16, kind="ExternalInput")
    w1_d = nc.dram_tensor("w1r", [128, 4, 2, 128], f16, kind="ExternalInput")
    wv_d = nc.dram_tensor("wvr", [128, 4, C], f16, kind="ExternalInput")
    wtop_d = nc.dram_tensor("wtopr", [128, H, 2, 2, 128], f16, kind="ExternalInput")
    wbot_d = nc.dram_tensor("wbotr", [128, H, 2, 2, 128], f16, kind="ExternalInput")
    pblk_d = nc.dram_tensor("pblkr", [128, 2 * H, H], f16, kind="ExternalInput")
    wcc_d = nc.dram_tensor("wccr", [128, 2 * H, OUTF], f16, kind="ExternalInput")
    bcc_d = nc.dram_tensor("bccr", [1, OUTF], f16, kind="ExternalInput")
    b1c_d = nc.dram_tensor("b1c", [128, 2], f32, kind="ExternalInput")
    id8_d = nc.dram_tensor("id8", [8, 8], f16, kind="ExternalInput")
    ones18_d = nc.dram_tensor("ones18", [1, 8], f16, kind="ExternalInput")
    out_d = nc.dram_tensor("out", [BLOC, OUTF], f32, kind="ExternalOutput")

    with tile.TileContext(nc) as tc, ExitStack() as stk:
        const = stk.enter_context(tc.tile_pool(name="const", bufs=1))
        xtp = stk.enter_context(tc.tile_pool(name="xtp", bufs=3))
        d3p = stk.enter_context(tc.tile_pool(name="d3p", bufs=3))
        tvpool = stk.enter_context(tc.tile_pool(name="tvpool", bufs=2))
        usp = stk.enter_context(tc.tile_pool(name="usp", bufs=4))
        smallsb = stk.enter_context(tc.tile_pool(name="smallsb", bufs=2))
        vpool = stk.enter_context(tc.tile_pool(name="vpool", bufs=1))
        pmm = stk.enter_context(tc.tile_pool(name="pmm", bufs=2, space="PSUM"))
        pzs = stk.enter_context(tc.tile_pool(name="pzs", bufs=3, space="PSUM"))
        patp = stk.enter_context(tc.tile_pool(name="patp", bufs=2, space="PSUM"))
        psmall = stk.enter_context(
            tc.tile_pool(name="psmall", bufs=1, space="PSUM"))

        # ---- constants + X prefetch, ordered so PE can start ASAP ----

        # XT loads: [f(4 tiles of 128), s=512] via the xbar DMA transpose
        # (sync queue only: DMA_TRANSPOSE occupies its issuing queue ~1.3us,
        # putting any on nc.scalar stalls the ACT tanh stream)
        def load_xt(b, name):
            xt = xtp.tile([128, 4, S], f16, tag="xt", name=name)
            for kf in range(4):
                nc.sync.dma_start_transpose(
                    out=xt[:, kf, :],
                    in_=d2_d[b, :, kf * 128:(kf + 1) * 128])
            return xt

        id8_sb = const.tile([8, 8], f16, tag="id8")
        nc.sync.dma_start(out=id8_sb, in_=id8_d[:, :])
        w1_sb = const.tile([128, 4, 2, 128], f16, tag="w1")
        nc.sync.dma_start(out=w1_sb, in_=w1_d[:, :, :, :])
        d1t_sb = const.tile([128, 4, BLOC], f16, tag="d1t")
        nc.sync.dma_start(out=d1t_sb, in_=d1t_d[:, :, :])
        b1c_sb = const.tile([128, 2], f32, tag="b1c")
        nc.sync.dma_start(out=b1c_sb, in_=b1c_d[:, :])
        wv_sb = const.tile([128, 4, C], f16, tag="wv")
        nc.sync.dma_start(out=wv_sb, in_=wv_d[:, :, :])
        ones18_sb = const.tile([1, 8], f16, tag="ones18")
        nc.sync.dma_start(out=ones18_sb, in_=ones18_d[:, :])
        bcc_sb = const.tile([1, OUTF], f16, tag="bcc")
        nc.sync.dma_start(out=bcc_sb, in_=bcc_d[:, :])

        # prefetch b=0 XT before the heavy weight DMAs
        xt0 = load_xt(0, "xt_pre0")

        wbot_sb = const.tile([128, H, 2, 2, 128], f16, tag="wbot")
        for h in range(H):
            nc.sync.dma_start(out=wbot_sb[:, h, :, :, :],
                              in_=wbot_d[:, h, :, :, :])
        pblk_sb = const.tile([128, 2 * H, H], f16, tag="pblk")
        nc.sync.dma_start(out=pblk_sb, in_=pblk_d[:, :, :])
        wtop_sb = const.tile([128, H, 2, 2, 128], f16, tag="wtop")
        for h in range(H):
            nc.sync.dma_start(out=wtop_sb[:, h, :, :, :],
                              in_=wtop_d[:, h, :, :, :])
        wcc_sb = const.tile([128, 2 * H, OUTF], f16, tag="wcc")

        # ---- d4T = relu(w1.T @ d1T + b1) : [C(2 tiles), BLOC] ----
        pd4 = psmall.tile([128, 2, BLOC], f32, tag="small")
        for m in range(2):
            for k in range(4):
                nc.tensor.matmul(pd4[:, m, :], lhsT=w1_sb[:, k, m, :],
                                 rhs=d1t_sb[:, k, :],
                                 start=(k == 0), stop=(k == 3))
        d4t_sb = const.tile([128, 2, BLOC], f16, tag="d4t")
        for m in range(2):
            nc.scalar.activation(d4t_sb[:, m, :], pd4[:, m, :], AF.Relu,
                                 bias=b1c_sb[:, m:m + 1])

        # ---- zconstT[h] = Wbot[h].T @ d4T : [2, C-tile, h, b] layout ----
        pzc = psmall.tile([128, 2, H, BLOC], f32, tag="small")
        for ct in range(2):
            for h in range(H):
                for ks in range(2):
                    nc.tensor.matmul(pzc[:, ct, h, :],
                                     lhsT=wbot_sb[:, h, ks, ct, :],
                                     rhs=d4t_sb[:, ks, :],
                                     start=(ks == 0), stop=(ks == 1))
        zc_sb = const.tile([128, 2, H, BLOC], f32, tag="zc")
        nc.vector.tensor_copy(out=zc_sb, in_=pzc)

        # ---- V accumulator across the b loop ----
        v_sb = vpool.tile([128, 2, H, BLOC], f16)  # [c-in-half, ch, h, b]

        for b in range(BLOC):
            # 1) XT via DMA transpose (b=0 prefetched above)
            xt = xt0 if b == 0 else load_xt(b, f"xt{b}")
            if b == 2:
                # wcc only needed for the final projection; load mid-stream
                nc.sync.dma_start(out=wcc_sb, in_=wcc_d[:, :, :])
            # 3) d3T = relu(w1.T @ XT + b1) [C(2), S]  (fp16 matmul)
            d3t = d3p.tile([128, 2, S], f16, tag="d3t", name=f"d3t{b}")
            for m in range(2):
                pmd3 = pmm.tile([128, S], f32, tag="mm", name=f"pmd3_{b}_{m}")
                for kf in range(4):
                    nc.tensor.matmul(pmd3, lhsT=w1_sb[:, kf, m, :],
                                     rhs=xt[:, kf, :],
                                     start=(kf == 0), stop=(kf == 3))
                nc.scalar.activation(d3t[:, m, :], pmd3, AF.Relu,
                                     bias=b1c_sb[:, m:m + 1])
            # 4) tv = tanh(X @ wv) [S(4), C] natural; 2 M-tiles per PSUM bank
            tv = tvpool.tile([128, 4, C], f16, tag="tv", name=f"tv{b}")
            for mp in range(2):
                pmtv = pmm.tile([128, 2, C], f32, tag="mm",
                                name=f"pmtv_{b}_{mp}")
                for ms2 in range(2):
                    ms = mp * 2 + ms2
                    for kf in range(4):
                        nc.tensor.matmul(
                            pmtv[:, ms2, :],
                            lhsT=xt[:, kf, ms * 128:(ms + 1) * 128],
                            rhs=wv_sb[:, kf, :],
                            start=(kf == 0), stop=(kf == 3))
                nc.scalar.activation(tv[:, mp * 2:(mp + 1) * 2, :], pmtv,
                                     AF.Tanh)
            # 5) per-head zs/us + atts accumulate
            pat = patp.tile([8, S], f32, tag="atts", name=f"pat{b}")
            for h in range(H):
                us = usp.tile([128, 2, S], f16, tag="us", name=f"us{b}_{h}")
                for ct in range(2):
                    pz = pzs.tile([128, S], f32, tag="zs",
                                  name=f"pz{b}_{h}_{ct}")
                    for ks in range(2):
                        nc.tensor.matmul(pz, lhsT=wtop_sb[:, h, ks, ct, :],
                                         rhs=d3t[:, ks, :],
                                         start=(ks == 0), stop=(ks == 1))
                    nc.scalar.activation(us[:, ct, :], pz, AF.Tanh,
                                         bias=zc_sb[:, ct, h, b:b + 1])
                for ct in range(2):
                    nc.tensor.matmul(pat, lhsT=pblk_sb[:, h * 2 + ct, :],
                                     rhs=us[:, ct, :],
                                     start=(h == 0 and ct == 0),
                                     stop=(h == H - 1 and ct == 1))
            # 6) softmax over S (normalization deferred to the vs eviction)
            nmax = smallsb.tile([8, 1], f32, tag="nmax", name=f"nmax{b}")
            nc.vector.tensor_reduce(nmax, pat, axis=mybir.AxisListType.X,
                                    op=mybir.AluOpType.max, negate=True)
            esc = smallsb.tile([8, S], f16, tag="esc", name=f"esc{b}")
            zsum = smallsb.tile([8, 1], f32, tag="zsum", name=f"zsum{b}")
            nc.scalar.activation(esc, pat, AF.Exp, bias=nmax, accum_out=zsum)
            zinv = smallsb.tile([8, 1], f32, tag="zinv", name=f"zinv{b}")
            nc.vector.reciprocal(zinv, zsum)
            # 7) scoresT (unnormalized) via PE transpose: [S(4 tiles), 8]
            psc = psmall.tile([128, 4, 8], f16, tag="small", name=f"psc{b}")
            for sc in range(4):
                nc.tensor.transpose(psc[:, sc, :],
                                    in_=esc[:, sc * 128:(sc + 1) * 128],
                                    identity=id8_sb)
            sct = smallsb.tile([128, 4, 8], f16, tag="sct", name=f"sct{b}")
            nc.vector.tensor_copy(out=sct, in_=psc)
            # 8) vs = scoresT.T @ tv : [8, C]; 1/Z applied at eviction
            pvs = psmall.tile([8, C], f32, tag="small", name=f"pvs{b}")
            for sc in range(4):
                nc.tensor.matmul(pvs, lhsT=sct[:, sc, :],
                                 rhs=tv[:, sc, :],
                                 start=(sc == 0), stop=(sc == 3))
            vssb = smallsb.tile([8, C], f16, tag="vssb", name=f"vssb{b}")
            nc.vector.tensor_scalar_mul(vssb, pvs, zinv)
            # 9) vsT into V slabs
            pvt = psmall.tile([128, 2, 8], f16, tag="small", name=f"pvt{b}")
            for ch in range(2):
                nc.tensor.transpose(pvt[:, ch, :],
                                    in_=vssb[:, ch * 128:(ch + 1) * 128],
                                    identity=id8_sb)
            for ch in range(2):
                nc.vector.tensor_copy(out=v_sb[:, ch, :, b:b + 1],
                                      in_=pvt[:, ch, :])

        # ---- final: out = relu(V.T @ wcc + bcc) ----
        pout = psmall.tile([8, OUTF], f32, tag="small")
        kidx = 0
        for h in range(H):
            for ch in range(2):
                nc.tensor.matmul(pout, lhsT=v_sb[:, ch, h, :],
                                 rhs=wcc_sb[:, h * 2 + ch, :],
                                 start=(kidx == 0), stop=False)
                kidx += 1
        nc.tensor.matmul(pout, lhsT=ones18_sb, rhs=bcc_sb,
                         start=False, stop=True)
        outsb = smallsb.tile([8, OUTF], f32, tag="outsb")
        nc.scalar.activation(outsb, pout, AF.Relu)
        nc.sync.dma_start(out=out_d[:, :], in_=outsb)

    nc.compile()
    return nc


def host_inputs(d1, d2, w1, b1, W, P, wv, wcc, bcc):
    """Host-side sharding + layout prep. Returns in_maps for 8 cores."""
    d1 = np.ascontiguousarray(d1, dtype=np.float32)
    d2 = np.ascontiguousarray(d2, dtype=np.float32)
    w1 = np.ascontiguousarray(w1, dtype=np.float32)
    b1 = np.ascontiguousarray(b1, dtype=np.float32)
    W = np.ascontiguousarray(W, dtype=np.float32)
    P = np.ascontiguousarray(P, dtype=np.float32)
    wv = np.ascontiguousarray(wv, dtype=np.float32)
    wcc = np.ascontiguousarray(wcc, dtype=np.float32)
    bcc = np.ascontiguousarray(bcc, dtype=np.float32)

    w1r = np.ascontiguousarray(
        w1.reshape(4, 128, 2, 128).transpose(1, 0, 2, 3))
    wvr = np.ascontiguousarray(wv.reshape(4, 128, C).transpose(1, 0, 2))
    wtopr = np.ascontiguousarray(
        W[:, :C, :].reshape(H, 2, 128, 2, 128).transpose(2, 0, 1, 3, 4))
    wbotr = np.ascontiguousarray(
        W[:, C:, :].reshape(H, 2, 128, 2, 128).transpose(2, 0, 1, 3, 4))
    pblkr = np.zeros((128, 2 * H, H), np.float32)
    for h in range(H):
        for ct in range(2):
            pblkr[:, h * 2 + ct, h] = P[h, ct * 128:(ct + 1) * 128]
    wccr = np.ascontiguousarray(
        wcc.reshape(2 * H, 128, OUTF).transpose(1, 0, 2))
    bccr = np.ascontiguousarray(bcc[None, :])
    b1c = np.ascontiguousarray(b1.reshape(2, 128).T)
    id8 = np.eye(8, dtype=np.float32)
    ones18 = np.ones((1, 8), np.float32)

    f16 = np.float16
    shared = dict(w1r=w1r.astype(f16), wvr=wvr.astype(f16),
                  wtopr=wtopr.astype(f16), wbotr=wbotr.astype(f16),
                  pblkr=pblkr.astype(f16), wccr=wccr.astype(f16),
                  bccr=bccr.astype(f16), b1c=b1c, id8=id8.astype(f16),
                  ones18=ones18.astype(f16))
    in_maps = []
    for core in range(NCORES):
        bs = slice(core * BLOC, (core + 1) * BLOC)
        d2c = np.ascontiguousarray(
            d2[:, bs, :].transpose(1, 0, 2).astype(np.float16))
        d1c = d1[bs]  # [BLOC, F]
        d1tr = np.ascontiguousarray(
            d1c.T.reshape(4, 128, BLOC).transpose(1, 0, 2)).astype(np.float16)
        in_maps.append(dict(d2=d2c, d1t=d1tr, **shared))
    return in_maps


def kernel(**inputs):
    if "nc" not in _CACHE:
        _CACHE["nc"] = build_nc()
    nc = _CACHE["nc"]
    in_maps = host_inputs(
        d1=inputs["d1"], d2=inputs["d2"], w1=inputs["w1"], b1=inputs["b1"],
        W=inputs["W"], P=inputs["P"], wv=inputs["wv"], wcc=inputs["wcc"],
        bcc=inputs["bcc"])
    from concourse.bass_utils import run_bass_kernel_spmd
    res = run_bass_kernel_spmd(nc, in_maps, core_ids=list(range(NCORES)))
    return np.concatenate([res.results[i]["out"] for i in range(NCORES)],
                          axis=0)



# revision 17
# speedup vs baseline: 1.0156x; 1.0156x over previous
"""Trainium2 Bass kernel for nn_Attention_63513976373985.

Strategy: pure data-parallel over the batch dim B=64 across 8 NeuronCores
(8 batches per core, all params replicated, no collectives).

v2 changes vs the DMA-transpose baseline (184.3us):
  - d2 is transposed on the HOST (numpy) and loaded with plain DMAs spread
    over 3 queues (sync/vector/gpsimd): kills 40us of serialized
    DMA_TRANSPOSE queue time and the ~30us cold start (HAM warmed only at
    t=32us before; PE now starts ~2us in and stays dense).
  - the P-reduction (atts = P . us, previously 16 matmuls/batch of N=512
    with only 8 of 128 PE rows used) is 4-way column-tiled: 4 concurrent
    accumulation chains in PSUM col-groups {0,32,64,96}, then one
    full-width [128,8] "comb" matmul folds the 4 groups into atts [8,S].
    ~4x less PE time for this stage.
  - d3 relu+bias moved to DVE (dual-op tensor_scalar); tv tanh fused into
    a single 1024-wide ACTIVATE: keeps ScalarE (the 2nd-busiest engine,
    ~125us busy in baseline) under the PE roofline.

Per-core pipeline per batch b (BLOC=8 batches):
  d3T = relu(w1.T @ XT + b1)        [C(2),S]   fp16 MMs, DVE relu
  tv  = tanh(X @ wv)                [S(4),C]   one 1024-wide tanh
  per head h: zs = Wtop[h].T @ d3T (+zconst bias in tanh ACT)  -> us
  atts via 4-way col-tiled block-diag P matvecs + comb matmul
  softmax over S (ACT exp w/ accum_out; 1/Z folded into vs eviction)
  scoresT via PE transpose, vs = scoresT.T @ tv  [H,C]
  V slabs via PE transpose, out = relu(V.T @ wcc + bcc)

Everything fp16 (fp32 PSUM accumulate).
"""
import sys

if "/opt/trn_rl_repo" not in sys.path:
    sys.path.insert(0, "/opt/trn_rl_repo")

import numpy as np

H, F, C, S, B = 8, 512, 256, 512, 64
NCORES = 8
BLOC = B // NCORES  # 8
OUTF = 128

_CACHE = {}


def build_nc(debug=False, dbg_b=0):
    import concourse.bass as bass  # noqa: F401
    import concourse.mybir as mybir
    import concourse.tile as tile
    from concourse import bacc
    from contextlib import ExitStack

    f32 = mybir.dt.float32
    f16 = mybir.dt.float16
    AF = mybir.ActivationFunctionType
    ALU = mybir.AluOpType

    nc = bacc.Bacc("TRN2", target_bir_lowering=False, debug=False,
                   num_devices=NCORES)

    # ---- DRAM parameters (per-core shard shapes) ----
    xt_d = nc.dram_tensor("xt", [128, 4, BLOC, S], f16, kind="ExternalInput")
    d1t_d = nc.dram_tensor("d1t", [128, 4, BLOC], f16, kind="ExternalInput")
    w1_d = nc.dram_tensor("w1r", [128, 4, 2, 128], f16, kind="ExternalInput")
    wv_d = nc.dram_tensor("wvr", [128, 4, C], f16, kind="ExternalInput")
    wtop_d = nc.dram_tensor("wtopr", [128, H, 2, 2, 128], f16,
                            kind="ExternalInput")
    wbot_d = nc.dram_tensor("wbotr", [128, H, 2, 2, 128], f16,
                            kind="ExternalInput")
    pblk_d = nc.dram_tensor("pblkr", [128, 2 * H, 4], f16,
                            kind="ExternalInput")
    comb_d = nc.dram_tensor("combr", [128, H], f16, kind="ExternalInput")
    wcc_d = nc.dram_tensor("wccr", [128, 2 * H, OUTF], f16,
                           kind="ExternalInput")
    bcc_d = nc.dram_tensor("bccr", [1, OUTF], f16, kind="ExternalInput")
    b1c_d = nc.dram_tensor("b1c", [128, 2], f32, kind="ExternalInput")
    id8_d = nc.dram_tensor("id8", [8, 8], f16, kind="ExternalInput")
    ones18_d = nc.dram_tensor("ones18", [1, 8], f16, kind="ExternalInput")
    out_d = nc.dram_tensor("out", [BLOC, OUTF], f32, kind="ExternalOutput")
    if debug:
        dbg_d3t = nc.dram_tensor("dbg_d3t", [128, 2, S], f16,
                                 kind="ExternalOutput")
        dbg_tv = nc.dram_tensor("dbg_tv", [128, 4, C], f16,
                                kind="ExternalOutput")
        dbg_attsg = nc.dram_tensor("dbg_attsg", [128, S], f16,
                                   kind="ExternalOutput")
        dbg_esc = nc.dram_tensor("dbg_esc", [8, S], f16,
                                 kind="ExternalOutput")
        dbg_zc = nc.dram_tensor("dbg_zc", [128, 2, H, BLOC], f32,
                                kind="ExternalOutput")
        dbg_us = nc.dram_tensor("dbg_us", [128, 2, S], f16,
                                kind="ExternalOutput")
        dbg_vs = nc.dram_tensor("dbg_vs", [BLOC, 8, C], f16,
                                kind="ExternalOutput")
        dbg_vsb = nc.dram_tensor("dbg_vsb", [128, 2, H, BLOC], f16,
                                 kind="ExternalOutput")

    with tile.TileContext(nc) as tc, ExitStack() as stk:
        const = stk.enter_context(tc.tile_pool(name="const", bufs=1))
        xtp = stk.enter_context(tc.tile_pool(name="xtp", bufs=BLOC))
        d3p = stk.enter_context(tc.tile_pool(name="d3p", bufs=2))
        tvpool = stk.enter_context(tc.tile_pool(name="tvpool", bufs=2))
        usp = stk.enter_context(tc.tile_pool(name="usp", bufs=4))
        smallsb = stk.enter_context(tc.tile_pool(name="smallsb", bufs=2))
        vpool = stk.enter_context(tc.tile_pool(name="vpool", bufs=1))
        pmm = stk.enter_context(tc.tile_pool(name="pmm", bufs=1, space="PSUM"))
        pzs = stk.enter_context(tc.tile_pool(name="pzs", bufs=2, space="PSUM"))
        patp = stk.enter_context(tc.tile_pool(name="patp", bufs=1,
                                              space="PSUM"))
        psmall = stk.enter_context(
            tc.tile_pool(name="psmall", bufs=1, space="PSUM"))

        # ---- DMA issue: 3 queues, ordered so PE can start ASAP ----
        # sync: small consts -> xt0 -> xt3 -> xt6 -> wcc
        id8_sb = const.tile([8, 8], f16, tag="id8")
        nc.sync.dma_start(out=id8_sb, in_=id8_d[:, :])
        b1c_sb = const.tile([128, 2], f32, tag="b1c")
        nc.sync.dma_start(out=b1c_sb, in_=b1c_d[:, :])
        ones18_sb = const.tile([1, 8], f16, tag="ones18")
        nc.sync.dma_start(out=ones18_sb, in_=ones18_d[:, :])
        bcc_sb = const.tile([1, OUTF], f16, tag="bcc")
        nc.sync.dma_start(out=bcc_sb, in_=bcc_d[:, :])
        d1t_sb = const.tile([128, 4, BLOC], f16, tag="d1t")
        nc.sync.dma_start(out=d1t_sb, in_=d1t_d[:, :, :])
        comb_sb = const.tile([128, H], f16, tag="comb")
        nc.sync.dma_start(out=comb_sb, in_=comb_d[:, :])
        pblk_sb = const.tile([128, 2 * H, 4], f16, tag="pblk")
        nc.sync.dma_start(out=pblk_sb, in_=pblk_d[:, :, :])
        w1_sb = const.tile([128, 4, 2, 128], f16, tag="w1")
        nc.sync.dma_start(out=w1_sb, in_=w1_d[:, :, :, :])

        xt_sb = []
        for b in range(BLOC):
            xt_sb.append(xtp.tile([128, 4, S], f16, tag="xt", name=f"xt{b}"))

        def load_xt(q, b):
            q.dma_start(out=xt_sb[b], in_=xt_d[:, :, b, :])

        load_xt(nc.sync, 0)

        # gpsimd queue: wv -> wbot -> wtop -> xt1/3/5/7
        wv_sb = const.tile([128, 4, C], f16, tag="wv")
        nc.gpsimd.dma_start(out=wv_sb, in_=wv_d[:, :, :])
        wbot_sb = const.tile([128, H, 2, 2, 128], f16, tag="wbot")
        for h in range(H):
            nc.gpsimd.dma_start(out=wbot_sb[:, h, :, :, :],
                                in_=wbot_d[:, h, :, :, :])
        wtop_sb = const.tile([128, H, 2, 2, 128], f16, tag="wtop")
        for h in range(H):
            nc.gpsimd.dma_start(out=wtop_sb[:, h, :, :, :],
                                in_=wtop_d[:, h, :, :, :])
        load_xt(nc.gpsimd, 1)
        load_xt(nc.gpsimd, 3)
        load_xt(nc.gpsimd, 5)
        load_xt(nc.gpsimd, 7)

        load_xt(nc.sync, 2)
        load_xt(nc.sync, 4)
        load_xt(nc.sync, 6)
        wcc_sb = const.tile([128, 2 * H, OUTF], f16, tag="wcc")
        nc.sync.dma_start(out=wcc_sb, in_=wcc_d[:, :, :])

        # ---- d4T = relu(w1.T @ d1T + b1) : [C(2 tiles), BLOC] ----
        pd4 = psmall.tile([128, 2, BLOC], f32, tag="small")
        for m in range(2):
            for k in range(4):
                nc.tensor.matmul(pd4[:, m, :], lhsT=w1_sb[:, k, m, :],
                                 rhs=d1t_sb[:, k, :],
                                 start=(k == 0), stop=(k == 3))
        d4t_sb = const.tile([128, 2, BLOC], f16, tag="d4t")
        for m in range(2):
            nc.scalar.activation(d4t_sb[:, m, :], pd4[:, m, :], AF.Relu,
                                 bias=b1c_sb[:, m:m + 1])

        # ---- atts accumulator bank: zero once (garbage rows stay 0) ----
        pat = patp.tile([128, S], f32, tag="atts")
        nc.vector.memset(pat[:, :], 0.0)

        # ---- V accumulator across the b loop ----
        v_sb = vpool.tile([128, 2, H, BLOC], f16)  # [c-in-half, ch, h, b]

        zc_sb = const.tile([128, 2, H, BLOC], f32, tag="zc")
        zc_done = False

        for b in range(BLOC):
            xt = xt_sb[b]
            # d3T = relu(w1.T @ XT + b1) [C(2), S]
            pmd3 = pmm.tile([128, 2, S], f32, tag="mm", name=f"pmd3_{b}")
            for m in range(2):
                for kf in range(4):
                    nc.tensor.matmul(pmd3[:, m, :], lhsT=w1_sb[:, kf, m, :],
                                     rhs=xt[:, kf, :],
                                     start=(kf == 0), stop=(kf == 3))
            d3t = d3p.tile([128, 2, S], f16, tag="d3t", name=f"d3t{b}")
            for m in range(2):
                nc.vector.tensor_scalar(
                    d3t[:, m, :], pmd3[:, m, :],
                    scalar1=b1c_sb[:, m:m + 1], scalar2=0.0,
                    op0=ALU.add, op1=ALU.max)

            # tv = tanh(X @ wv) [S(4 tiles), C]; single 2-bank PSUM tile
            pmtv = pmm.tile([128, 4, C], f32, tag="mm", name=f"pmtv_{b}")
            for sc in range(4):
                for kf in range(4):
                    nc.tensor.matmul(
                        pmtv[:, sc, :],
                        lhsT=xt[:, kf, sc * 128:(sc + 1) * 128],
                        rhs=wv_sb[:, kf, :],
                        start=(kf == 0), stop=(kf == 3))
            tv = tvpool.tile([128, 4, C], f16, tag="tv", name=f"tv{b}")
            nc.scalar.activation(tv[:, :, :], pmtv[:, :, :], AF.Tanh)

            if not zc_done:
                # zconstT[h] = Wbot[h].T @ d4T ; placed after b0's d3/tv so
                # the PE queue never stalls waiting for the wbot DMA.
                zc_done = True
                pzc = psmall.tile([128, 2, H, BLOC], f32, tag="small")
                for ct in range(2):
                    for h in range(H):
                        for ks in range(2):
                            nc.tensor.matmul(pzc[:, ct, h, :],
                                             lhsT=wbot_sb[:, h, ks, ct, :],
                                             rhs=d4t_sb[:, ks, :],
                                             start=(ks == 0), stop=(ks == 1))
                nc.vector.tensor_copy(out=zc_sb, in_=pzc)
                if debug:
                    nc.sync.dma_start(out=dbg_zc[:, :, :, :], in_=zc_sb)

            # per-head us + 4-way col-tiled atts accumulation
            for h in range(H):
                pz = pzs.tile([128, 2, S], f32, tag="zs", name=f"pz{b}_{h}")
                for ct in range(2):
                    for ks in range(2):
                        nc.tensor.matmul(pz[:, ct, :],
                                         lhsT=wtop_sb[:, h, ks, ct, :],
                                         rhs=d3t[:, ks, :],
                                         start=(ks == 0), stop=(ks == 1))
                us = usp.tile([128, 2, S], f16, tag="us", name=f"us{b}_{h}")
                for ct in range(2):
                    nc.scalar.activation(us[:, ct, :], pz[:, ct, :], AF.Tanh,
                                         bias=zc_sb[:, ct, h, b:b + 1])
                if debug and b == dbg_b and h == 0:
                    nc.sync.dma_start(out=dbg_us[:, :, :], in_=us)
                for ct in range(2):
                    g = 2 * (h % 2) + ct
                    # The has_written clear of start=True covers only the
                    # bank rows of the partitions this matmul addresses
                    # (HW-verified: a single bank-wide start leaves other
                    # groups accumulating onto the previous batch), so each
                    # col-group chain carries its own start/stop.
                    nc.tensor.matmul(pat[32 * g:32 * g + 4, :],
                                     lhsT=pblk_sb[:, h * 2 + ct, :],
                                     rhs=us[:, ct, :],
                                     start=(h < 2), stop=(h >= 6),
                                     skip_group_check=True,
                                     tile_position=(0, 32 * g))

            # fold the 4 col-groups: atts = comb.T @ pat  [8, S]
            attsg = smallsb.tile([128, S], f16, tag="attsg", name=f"ag{b}")
            nc.vector.tensor_copy(out=attsg, in_=pat)
            if debug and b == dbg_b:
                nc.sync.dma_start(out=dbg_d3t[:, :, :], in_=d3t)
                nc.sync.dma_start(out=dbg_tv[:, :, :], in_=tv)
                nc.sync.dma_start(out=dbg_attsg[:, :], in_=attsg)
            pat2 = psmall.tile([8, S], f32, tag="small", name=f"pat2_{b}")
            nc.tensor.matmul(pat2, lhsT=comb_sb, rhs=attsg,
                             start=True, stop=True)

            # softmax over S (normalization deferred to the vs eviction)
            nmax = smallsb.tile([8, 1], f32, tag="nmax", name=f"nmax{b}")
            nc.vector.tensor_reduce(nmax, pat2, axis=mybir.AxisListType.X,
                                    op=ALU.max, negate=True)
            esc = smallsb.tile([8, S], f16, tag="esc", name=f"esc{b}")
            zsum = smallsb.tile([8, 1], f32, tag="zsum", name=f"zsum{b}")
            nc.scalar.activation(esc, pat2, AF.Exp, bias=nmax, accum_out=zsum)
            zinv = smallsb.tile([8, 1], f32, tag="zinv", name=f"zinv{b}")
            nc.vector.reciprocal(zinv, zsum)
            if debug and b == dbg_b:
                nc.sync.dma_start(out=dbg_esc[:, :], in_=esc)

            # scoresT (unnormalized) via PE transpose: [S(4 tiles), 8]
            psc = psmall.tile([128, 4, 8], f16, tag="small", name=f"psc{b}")
            for sc in range(4):
                nc.tensor.transpose(psc[:, sc, :],
                                    in_=esc[:, sc * 128:(sc + 1) * 128],
                                    identity=id8_sb)
            sct = smallsb.tile([128, 4, 8], f16, tag="sct", name=f"sct{b}")
            nc.vector.tensor_copy(out=sct, in_=psc)
            # vs = scoresT.T @ tv : [8, C]; 1/Z applied at eviction
            pvs = psmall.tile([8, C], f32, tag="small", name=f"pvs{b}")
            for sc in range(4):
                nc.tensor.matmul(pvs, lhsT=sct[:, sc, :],
                                 rhs=tv[:, sc, :],
                                 start=(sc == 0), stop=(sc == 3))
            vssb = smallsb.tile([8, C], f16, tag="vssb", name=f"vssb{b}")
            nc.vector.tensor_scalar_mul(vssb, pvs, zinv)
            if debug:
                nc.sync.dma_start(out=dbg_vs[b, :, :], in_=vssb)
            # vsT into V slabs
            pvt = psmall.tile([128, 2, 8], f16, tag="small", name=f"pvt{b}")
            for ch in range(2):
                nc.tensor.transpose(pvt[:, ch, :],
                                    in_=vssb[:, ch * 128:(ch + 1) * 128],
                                    identity=id8_sb)
            for ch in range(2):
                nc.vector.tensor_copy(out=v_sb[:, ch, :, b:b + 1],
                                      in_=pvt[:, ch, :])

        if debug:
            nc.sync.dma_start(out=dbg_vsb[:, :, :, :], in_=v_sb)
        # ---- final: out = relu(V.T @ wcc + bcc) ----
        pout = psmall.tile([8, OUTF], f32, tag="small")
        kidx = 0
        for h in range(H):
            for ch in range(2):
                nc.tensor.matmul(pout, lhsT=v_sb[:, ch, h, :],
                                 rhs=wcc_sb[:, h * 2 + ch, :],
                                 start=(kidx == 0), stop=False)
                kidx += 1
        nc.tensor.matmul(pout, lhsT=ones18_sb, rhs=bcc_sb,
                         start=False, stop=True)
        outsb = smallsb.tile([8, OUTF], f32, tag="outsb")
        nc.scalar.activation(outsb, pout, AF.Relu)
        nc.sync.dma_start(out=out_d[:, :], in_=outsb)

    nc.compile()
    return nc


def host_inputs(d1, d2, w1, b1, W, P, wv, wcc, bcc):
    """Host-side sharding + layout prep. Returns in_maps for 8 cores."""
    d1 = np.ascontiguousarray(d1, dtype=np.float32)
    d2 = np.ascontiguousarray(d2, dtype=np.float32)
    w1 = np.ascontiguousarray(w1, dtype=np.float32)
    b1 = np.ascontiguousarray(b1, dtype=np.float32)
    W = np.ascontiguousarray(W, dtype=np.float32)
    P = np.ascontiguousarray(P, dtype=np.float32)
    wv = np.ascontiguousarray(wv, dtype=np.float32)
    wcc = np.ascontiguousarray(wcc, dtype=np.float32)
    bcc = np.ascontiguousarray(bcc, dtype=np.float32)

    w1r = np.ascontiguousarray(
        w1.reshape(4, 128, 2, 128).transpose(1, 0, 2, 3))
    wvr = np.ascontiguousarray(wv.reshape(4, 128, C).transpose(1, 0, 2))
    wtopr = np.ascontiguousarray(
        W[:, :C, :].reshape(H, 2, 128, 2, 128).transpose(2, 0, 1, 3, 4))
    wbotr = np.ascontiguousarray(
        W[:, C:, :].reshape(H, 2, 128, 2, 128).transpose(2, 0, 1, 3, 4))
    # 4-way col-tiled P blocks: head h, half ct -> col-group g=2*(h%2)+ct,
    # output row r=h//2 within the group.
    pblkr = np.zeros((128, 2 * H, 4), np.float32)
    combr = np.zeros((128, H), np.float32)
    for h in range(H):
        r = h // 2
        for ct in range(2):
            g = 2 * (h % 2) + ct
            pblkr[:, h * 2 + ct, r] = P[h, ct * 128:(ct + 1) * 128]
            combr[32 * g + r, h] = 1.0
    wccr = np.ascontiguousarray(
        wcc.reshape(2 * H, 128, OUTF).transpose(1, 0, 2))
    bccr = np.ascontiguousarray(bcc[None, :])
    b1c = np.ascontiguousarray(b1.reshape(2, 128).T)
    id8 = np.eye(8, dtype=np.float32)
    ones18 = np.ones((1, 8), np.float32)

    f16 = np.float16
    shared = dict(w1r=w1r.astype(f16), wvr=wvr.astype(f16),
                  wtopr=wtopr.astype(f16), wbotr=wbotr.astype(f16),
                  pblkr=pblkr.astype(f16), combr=combr.astype(f16),
                  wccr=wccr.astype(f16),
                  bccr=bccr.astype(f16), b1c=b1c, id8=id8.astype(f16),
                  ones18=ones18.astype(f16))
    in_maps = []
    for core in range(NCORES):
        bs = slice(core * BLOC, (core + 1) * BLOC)
        # xt[p, kf, b, s] = d2[s, bs.start+b, kf*128+p]
        d2c = d2[:, bs, :]  # [S, BLOC, F]
        xtr = np.ascontiguousarray(
            d2c.transpose(2, 1, 0).reshape(4, 128, BLOC, S)
            .transpose(1, 0, 2, 3)).astype(np.float16)
        d1c = d1[bs]  # [BLOC, F]
        d1tr = np.ascontiguousarray(
            d1c.T.reshape(4, 128, BLOC).transpose(1, 0, 2)).astype(np.float16)
        in_maps.append(dict(xt=xtr, d1t=d1tr, **shared))
    return in_maps


def kernel(**inputs):
    if "nc" not in _CACHE:
        _CACHE["nc"] = build_nc()
    nc = _CACHE["nc"]
    in_maps = host_inputs(
        d1=inputs["d1"], d2=inputs["d2"], w1=inputs["w1"], b1=inputs["b1"],
        W=inputs["W"], P=inputs["P"], wv=inputs["wv"], wcc=inputs["wcc"],
        bcc=inputs["bcc"])
    from concourse.bass_utils import run_bass_kernel_spmd
    res = run_bass_kernel_spmd(nc, in_maps, core_ids=list(range(NCORES)))
    return np.concatenate([res.results[i]["out"] for i in range(NCORES)],
                          axis=0)


# revision 18
# speedup vs baseline: 1.1462x; 1.1286x over previous
"""Trainium2 Bass kernel for nn_Attention_63513976373985.

Strategy: pure data-parallel over the batch dim B=64 across 8 NeuronCores
(8 batches per core, all params replicated, no collectives).

v3: software-pipelined schedule.
  - d2 transposed on the HOST, loaded with plain DMAs on 2 queues (no
    xbar DMA-transposes, PE starts ~2us in).
  - P-reduction (atts = P . us) is 4-way column-tiled: pat matmuls are
    emitted as 4-MM quads (heads 2k,2k+1 x ct) delayed two heads behind
    the zs/tanh producer so all four col-groups run concurrently on the
    PE with their inputs already in SBUF; a full-width [128,8] "comb"
    matmul folds the groups into atts [8,S]. Each group chain carries its
    own start/stop (the has_written clear of start=True covers only the
    addressed partition rows -- HW-verified).
  - batch b-1's softmax/vs tail is interleaved into batch b's head loop
    (exp under h=1, score-transposes under h=2, vs matmuls under h=3,
    V-slab transposes under h=4) so the PE never idles through the
    softmax latency chain and HAM stays at K=8/8.
  - d3 relu+bias on DVE (dual-op tensor_scalar); tv tanh one 1024-wide
    ACTIVATE; exp with accum_out; 1/Z folded into the vs eviction.

Everything fp16 (fp32 PSUM accumulate).
"""
import sys

if "/opt/trn_rl_repo" not in sys.path:
    sys.path.insert(0, "/opt/trn_rl_repo")

import numpy as np

H, F, C, S, B = 8, 512, 256, 512, 64
NCORES = 8
BLOC = B // NCORES  # 8
OUTF = 128

_CACHE = {}


def build_nc(debug=False, dbg_b=0):
    import concourse.bass as bass  # noqa: F401
    import concourse.mybir as mybir
    import concourse.tile as tile
    from concourse import bacc
    from contextlib import ExitStack

    f32 = mybir.dt.float32
    f16 = mybir.dt.float16
    AF = mybir.ActivationFunctionType
    ALU = mybir.AluOpType

    nc = bacc.Bacc("TRN2", target_bir_lowering=False, debug=False,
                   num_devices=NCORES)

    # ---- DRAM parameters (per-core shard shapes) ----
    xt_d = nc.dram_tensor("xt", [128, 4, BLOC, S], f16, kind="ExternalInput")
    d1t_d = nc.dram_tensor("d1t", [128, 4, BLOC], f16, kind="ExternalInput")
    w1_d = nc.dram_tensor("w1r", [128, 4, 2, 128], f16, kind="ExternalInput")
    wv_d = nc.dram_tensor("wvr", [128, 4, C], f16, kind="ExternalInput")
    wtop_d = nc.dram_tensor("wtopr", [128, H, 2, 2, 128], f16,
                            kind="ExternalInput")
    wbot_d = nc.dram_tensor("wbotr", [128, H, 2, 2, 128], f16,
                            kind="ExternalInput")
    pblk_d = nc.dram_tensor("pblkr", [128, 2 * H, 4], f16,
                            kind="ExternalInput")
    comb_d = nc.dram_tensor("combr", [128, H], f16, kind="ExternalInput")
    wcc_d = nc.dram_tensor("wccr", [128, 2 * H, OUTF], f16,
                           kind="ExternalInput")
    bcc_d = nc.dram_tensor("bccr", [1, OUTF], f16, kind="ExternalInput")
    b1c_d = nc.dram_tensor("b1c", [128, 2], f32, kind="ExternalInput")
    id8_d = nc.dram_tensor("id8", [8, 8], f16, kind="ExternalInput")
    ones18_d = nc.dram_tensor("ones18", [1, 8], f16, kind="ExternalInput")
    out_d = nc.dram_tensor("out", [BLOC, OUTF], f32, kind="ExternalOutput")
    if debug:
        dbg_d3t = nc.dram_tensor("dbg_d3t", [128, 2, S], f16,
                                 kind="ExternalOutput")
        dbg_tv = nc.dram_tensor("dbg_tv", [128, 4, C], f16,
                                kind="ExternalOutput")
        dbg_attsg = nc.dram_tensor("dbg_attsg", [128, S], f16,
                                   kind="ExternalOutput")
        dbg_esc = nc.dram_tensor("dbg_esc", [8, S], f16,
                                 kind="ExternalOutput")
        dbg_zc = nc.dram_tensor("dbg_zc", [128, 2, H, BLOC], f32,
                                kind="ExternalOutput")
        dbg_vs = nc.dram_tensor("dbg_vs", [BLOC, 8, C], f16,
                                kind="ExternalOutput")

    with tile.TileContext(nc) as tc, ExitStack() as stk:
        const = stk.enter_context(tc.tile_pool(name="const", bufs=1))
        xtp = stk.enter_context(tc.tile_pool(name="xtp", bufs=BLOC))
        d3p = stk.enter_context(tc.tile_pool(name="d3p", bufs=2))
        tvpool = stk.enter_context(tc.tile_pool(name="tvpool", bufs=2))
        usp = stk.enter_context(tc.tile_pool(name="usp", bufs=4))
        smallsb = stk.enter_context(tc.tile_pool(name="smallsb", bufs=2))
        vpool = stk.enter_context(tc.tile_pool(name="vpool", bufs=1))
        pmm = stk.enter_context(tc.tile_pool(name="pmm", bufs=1, space="PSUM"))
        pzs = stk.enter_context(tc.tile_pool(name="pzs", bufs=2, space="PSUM"))
        patp = stk.enter_context(tc.tile_pool(name="patp", bufs=1,
                                              space="PSUM"))
        psmall = stk.enter_context(
            tc.tile_pool(name="psmall", bufs=1, space="PSUM"))

        # ---- DMA issue: 2 queues, ordered so PE can start ASAP ----
        id8_sb = const.tile([8, 8], f16, tag="id8")
        nc.sync.dma_start(out=id8_sb, in_=id8_d[:, :])
        b1c_sb = const.tile([128, 2], f32, tag="b1c")
        nc.sync.dma_start(out=b1c_sb, in_=b1c_d[:, :])
        ones18_sb = const.tile([1, 8], f16, tag="ones18")
        nc.sync.dma_start(out=ones18_sb, in_=ones18_d[:, :])
        bcc_sb = const.tile([1, OUTF], f16, tag="bcc")
        nc.sync.dma_start(out=bcc_sb, in_=bcc_d[:, :])
        d1t_sb = const.tile([128, 4, BLOC], f16, tag="d1t")
        nc.sync.dma_start(out=d1t_sb, in_=d1t_d[:, :, :])
        comb_sb = const.tile([128, H], f16, tag="comb")
        nc.sync.dma_start(out=comb_sb, in_=comb_d[:, :])
        pblk_sb = const.tile([128, 2 * H, 4], f16, tag="pblk")
        nc.sync.dma_start(out=pblk_sb, in_=pblk_d[:, :, :])
        w1_sb = const.tile([128, 4, 2, 128], f16, tag="w1")
        nc.sync.dma_start(out=w1_sb, in_=w1_d[:, :, :, :])

        xt_sb = []
        for b in range(BLOC):
            xt_sb.append(xtp.tile([128, 4, S], f16, tag="xt", name=f"xt{b}"))

        def load_xt(q, b):
            q.dma_start(out=xt_sb[b], in_=xt_d[:, :, b, :])

        load_xt(nc.sync, 0)

        # gpsimd queue: wv -> wbot -> wtop -> xt odd
        wv_sb = const.tile([128, 4, C], f16, tag="wv")
        nc.gpsimd.dma_start(out=wv_sb, in_=wv_d[:, :, :])
        wbot_sb = const.tile([128, H, 2, 2, 128], f16, tag="wbot")
        for h in range(H):
            nc.gpsimd.dma_start(out=wbot_sb[:, h, :, :, :],
                                in_=wbot_d[:, h, :, :, :])
        wtop_sb = const.tile([128, H, 2, 2, 128], f16, tag="wtop")
        for h in range(H):
            nc.gpsimd.dma_start(out=wtop_sb[:, h, :, :, :],
                                in_=wtop_d[:, h, :, :, :])
        load_xt(nc.gpsimd, 1)
        load_xt(nc.gpsimd, 3)
        load_xt(nc.gpsimd, 5)
        load_xt(nc.gpsimd, 7)

        load_xt(nc.sync, 2)
        load_xt(nc.sync, 4)
        load_xt(nc.sync, 6)
        wcc_sb = const.tile([128, 2 * H, OUTF], f16, tag="wcc")
        nc.sync.dma_start(out=wcc_sb, in_=wcc_d[:, :, :])

        # ---- d4T = relu(w1.T @ d1T + b1) : [C(2 tiles), BLOC] ----
        pd4 = psmall.tile([128, 2, BLOC], f32, tag="small")
        for m in range(2):
            for k in range(4):
                nc.tensor.matmul(pd4[:, m, :], lhsT=w1_sb[:, k, m, :],
                                 rhs=d1t_sb[:, k, :],
                                 start=(k == 0), stop=(k == 3))
        d4t_sb = const.tile([128, 2, BLOC], f16, tag="d4t")
        for m in range(2):
            nc.scalar.activation(d4t_sb[:, m, :], pd4[:, m, :], AF.Relu,
                                 bias=b1c_sb[:, m:m + 1])

        # ---- atts accumulator bank: zero once (garbage rows stay 0) ----
        pat = patp.tile([128, S], f32, tag="atts")
        nc.vector.memset(pat[:, :], 0.0)

        v_sb = vpool.tile([128, 2, H, BLOC], f16)  # [c-in-half, ch, h, b]
        zc_sb = const.tile([128, 2, H, BLOC], f32, tag="zc")

        # ---- pipeline stage emitters ----
        d3ts = [None] * BLOC
        tvs = [None] * BLOC
        uss = {}
        pat2s = [None] * BLOC
        nmaxs = [None] * BLOC
        escs = [None] * BLOC
        zinvs = [None] * BLOC
        scts = [None] * BLOC
        vssbs = [None] * BLOC

        def emit_d3_tv(b):
            xt = xt_sb[b]
            pmd3 = pmm.tile([128, 2, S], f32, tag="mm", name=f"pmd3_{b}")
            for m in range(2):
                for kf in range(4):
                    nc.tensor.matmul(pmd3[:, m, :], lhsT=w1_sb[:, kf, m, :],
                                     rhs=xt[:, kf, :],
                                     start=(kf == 0), stop=(kf == 3))
            d3t = d3p.tile([128, 2, S], f16, tag="d3t", name=f"d3t{b}")
            for m in range(2):
                nc.vector.tensor_scalar(
                    d3t[:, m, :], pmd3[:, m, :],
                    scalar1=b1c_sb[:, m:m + 1], scalar2=0.0,
                    op0=ALU.add, op1=ALU.max)
            d3ts[b] = d3t
            pmtv = pmm.tile([128, 4, C], f32, tag="mm", name=f"pmtv_{b}")
            for sc in range(4):
                for kf in range(4):
                    nc.tensor.matmul(
                        pmtv[:, sc, :],
                        lhsT=xt[:, kf, sc * 128:(sc + 1) * 128],
                        rhs=wv_sb[:, kf, :],
                        start=(kf == 0), stop=(kf == 3))
            tv = tvpool.tile([128, 4, C], f16, tag="tv", name=f"tv{b}")
            nc.scalar.activation(tv[:, :, :], pmtv[:, :, :], AF.Tanh)
            tvs[b] = tv
            if debug and b == dbg_b:
                nc.sync.dma_start(out=dbg_d3t[:, :, :], in_=d3t)
                nc.sync.dma_start(out=dbg_tv[:, :, :], in_=tv)

        def emit_zc():
            pzc = psmall.tile([128, 2, H, BLOC], f32, tag="small")
            for ct in range(2):
                for h in range(H):
                    for ks in range(2):
                        nc.tensor.matmul(pzc[:, ct, h, :],
                                         lhsT=wbot_sb[:, h, ks, ct, :],
                                         rhs=d4t_sb[:, ks, :],
                                         start=(ks == 0), stop=(ks == 1))
            nc.vector.tensor_copy(out=zc_sb, in_=pzc)
            if debug:
                nc.sync.dma_start(out=dbg_zc[:, :, :, :], in_=zc_sb)

        def emit_zs(b, h):
            pz = pzs.tile([128, 2, S], f32, tag="zs", name=f"pz{b}_{h}")
            for ct in range(2):
                for ks in range(2):
                    nc.tensor.matmul(pz[:, ct, :],
                                     lhsT=wtop_sb[:, h, ks, ct, :],
                                     rhs=d3ts[b][:, ks, :],
                                     start=(ks == 0), stop=(ks == 1))
            us = usp.tile([128, 2, S], f16, tag="us", name=f"us{b}_{h}")
            for ct in range(2):
                nc.scalar.activation(us[:, ct, :], pz[:, ct, :], AF.Tanh,
                                     bias=zc_sb[:, ct, h, b:b + 1])
            uss[(b, h)] = us

        def emit_pat_quad(b, k):
            # 4 adjacent matmuls covering all 4 col-groups -> concurrent.
            for hh in (2 * k, 2 * k + 1):
                us = uss.pop((b, hh))
                for ct in range(2):
                    g = 2 * (hh % 2) + ct
                    nc.tensor.matmul(pat[32 * g:32 * g + 4, :],
                                     lhsT=pblk_sb[:, hh * 2 + ct, :],
                                     rhs=us[:, ct, :],
                                     start=(hh < 2), stop=(hh >= 6),
                                     skip_group_check=True,
                                     tile_position=(0, 32 * g))

        def emit_softmax_pre(b):
            attsg = smallsb.tile([128, S], f16, tag="attsg", name=f"ag{b}")
            nc.vector.tensor_copy(out=attsg, in_=pat)
            if debug and b == dbg_b:
                nc.sync.dma_start(out=dbg_attsg[:, :], in_=attsg)
            pat2 = psmall.tile([8, S], f32, tag="small", name=f"pat2_{b}")
            nc.tensor.matmul(pat2, lhsT=comb_sb, rhs=attsg,
                             start=True, stop=True)
            nmax = smallsb.tile([8, 1], f32, tag="nmax", name=f"nmax{b}")
            nc.vector.tensor_reduce(nmax, pat2, axis=mybir.AxisListType.X,
                                    op=ALU.max, negate=True)
            pat2s[b] = pat2
            nmaxs[b] = nmax

        def emit_softmax_exp(b):
            esc = smallsb.tile([8, S], f16, tag="esc", name=f"esc{b}")
            zsum = smallsb.tile([8, 1], f32, tag="zsum", name=f"zsum{b}")
            nc.scalar.activation(esc, pat2s[b], AF.Exp, bias=nmaxs[b],
                                 accum_out=zsum)
            zinv = smallsb.tile([8, 1], f32, tag="zinv", name=f"zinv{b}")
            nc.vector.reciprocal(zinv, zsum)
            escs[b] = esc
            zinvs[b] = zinv
            if debug and b == dbg_b:
                nc.sync.dma_start(out=dbg_esc[:, :], in_=esc)

        def emit_tail_psc(b):
            psc = psmall.tile([128, 4, 8], f16, tag="small", name=f"psc{b}")
            for sc in range(4):
                nc.tensor.transpose(psc[:, sc, :],
                                    in_=escs[b][:, sc * 128:(sc + 1) * 128],
                                    identity=id8_sb)
            sct = smallsb.tile([128, 4, 8], f16, tag="sct", name=f"sct{b}")
            nc.vector.tensor_copy(out=sct, in_=psc)
            scts[b] = sct

        def emit_tail_vs(b):
            pvs = psmall.tile([8, C], f32, tag="small", name=f"pvs{b}")
            for sc in range(4):
                nc.tensor.matmul(pvs, lhsT=scts[b][:, sc, :],
                                 rhs=tvs[b][:, sc, :],
                                 start=(sc == 0), stop=(sc == 3))
            vssb = smallsb.tile([8, C], f16, tag="vssb", name=f"vssb{b}")
            nc.vector.tensor_scalar_mul(vssb, pvs, zinvs[b])
            vssbs[b] = vssb
            if debug:
                nc.sync.dma_start(out=dbg_vs[b, :, :], in_=vssb)

        def emit_tail_pvt(b):
            pvt = psmall.tile([128, 2, 8], f16, tag="small", name=f"pvt{b}")
            for ch in range(2):
                nc.tensor.transpose(
                    pvt[:, ch, :],
                    in_=vssbs[b][:, ch * 128:(ch + 1) * 128],
                    identity=id8_sb)
            for ch in range(2):
                nc.vector.tensor_copy(out=v_sb[:, ch, :, b:b + 1],
                                      in_=pvt[:, ch, :])

        # ---- pipelined emission ----
        emit_d3_tv(0)
        emit_zc()
        for b in range(BLOC):
            for h in range(H):
                emit_zs(b, h)
                if h == 2:
                    emit_pat_quad(b, 0)
                if h == 4:
                    emit_pat_quad(b, 1)
                if h == 6:
                    emit_pat_quad(b, 2)
                if b > 0:
                    if h == 0:
                        emit_softmax_pre(b - 1)
                    elif h == 1:
                        emit_softmax_exp(b - 1)
                    elif h == 2:
                        emit_tail_psc(b - 1)
                    elif h == 3:
                        emit_tail_vs(b - 1)
                    elif h == 4:
                        emit_tail_pvt(b - 1)
            if b + 1 < BLOC:
                emit_d3_tv(b + 1)
            emit_pat_quad(b, 3)
        emit_softmax_pre(BLOC - 1)
        emit_softmax_exp(BLOC - 1)
        emit_tail_psc(BLOC - 1)
        emit_tail_vs(BLOC - 1)
        emit_tail_pvt(BLOC - 1)

        # ---- final: out = relu(V.T @ wcc + bcc) ----
        pout = psmall.tile([8, OUTF], f32, tag="small")
        kidx = 0
        for h in range(H):
            for ch in range(2):
                nc.tensor.matmul(pout, lhsT=v_sb[:, ch, h, :],
                                 rhs=wcc_sb[:, h * 2 + ch, :],
                                 start=(kidx == 0), stop=False)
                kidx += 1
        nc.tensor.matmul(pout, lhsT=ones18_sb, rhs=bcc_sb,
                         start=False, stop=True)
        outsb = smallsb.tile([8, OUTF], f32, tag="outsb")
        nc.scalar.activation(outsb, pout, AF.Relu)
        nc.sync.dma_start(out=out_d[:, :], in_=outsb)

    nc.compile()
    return nc


def host_inputs(d1, d2, w1, b1, W, P, wv, wcc, bcc):
    """Host-side sharding + layout prep. Returns in_maps for 8 cores."""
    d1 = np.ascontiguousarray(d1, dtype=np.float32)
    d2 = np.ascontiguousarray(d2, dtype=np.float32)
    w1 = np.ascontiguousarray(w1, dtype=np.float32)
    b1 = np.ascontiguousarray(b1, dtype=np.float32)
    W = np.ascontiguousarray(W, dtype=np.float32)
    P = np.ascontiguousarray(P, dtype=np.float32)
    wv = np.ascontiguousarray(wv, dtype=np.float32)
    wcc = np.ascontiguousarray(wcc, dtype=np.float32)
    bcc = np.ascontiguousarray(bcc, dtype=np.float32)

    w1r = np.ascontiguousarray(
        w1.reshape(4, 128, 2, 128).transpose(1, 0, 2, 3))
    wvr = np.ascontiguousarray(wv.reshape(4, 128, C).transpose(1, 0, 2))
    wtopr = np.ascontiguousarray(
        W[:, :C, :].reshape(H, 2, 128, 2, 128).transpose(2, 0, 1, 3, 4))
    wbotr = np.ascontiguousarray(
        W[:, C:, :].reshape(H, 2, 128, 2, 128).transpose(2, 0, 1, 3, 4))
    # 4-way col-tiled P blocks: head h, half ct -> col-group g=2*(h%2)+ct,
    # output row r=h//2 within the group.
    pblkr = np.zeros((128, 2 * H, 4), np.float32)
    combr = np.zeros((128, H), np.float32)
    for h in range(H):
        r = h // 2
        for ct in range(2):
            g = 2 * (h % 2) + ct
            pblkr[:, h * 2 + ct, r] = P[h, ct * 128:(ct + 1) * 128]
            combr[32 * g + r, h] = 1.0
    wccr = np.ascontiguousarray(
        wcc.reshape(2 * H, 128, OUTF).transpose(1, 0, 2))
    bccr = np.ascontiguousarray(bcc[None, :])
    b1c = np.ascontiguousarray(b1.reshape(2, 128).T)
    id8 = np.eye(8, dtype=np.float32)
    ones18 = np.ones((1, 8), np.float32)

    f16 = np.float16
    shared = dict(w1r=w1r.astype(f16), wvr=wvr.astype(f16),
                  wtopr=wtopr.astype(f16), wbotr=wbotr.astype(f16),
                  pblkr=pblkr.astype(f16), combr=combr.astype(f16),
                  wccr=wccr.astype(f16),
                  bccr=bccr.astype(f16), b1c=b1c, id8=id8.astype(f16),
                  ones18=ones18.astype(f16))
    in_maps = []
    for core in range(NCORES):
        bs = slice(core * BLOC, (core + 1) * BLOC)
        # xt[p, kf, b, s] = d2[s, bs.start+b, kf*128+p]
        d2c = d2[:, bs, :]  # [S, BLOC, F]
        xtr = np.ascontiguousarray(
            d2c.transpose(2, 1, 0).reshape(4, 128, BLOC, S)
            .transpose(1, 0, 2, 3)).astype(np.float16)
        d1c = d1[bs]  # [BLOC, F]
        d1tr = np.ascontiguousarray(
            d1c.T.reshape(4, 128, BLOC).transpose(1, 0, 2)).astype(np.float16)
        in_maps.append(dict(xt=xtr, d1t=d1tr, **shared))
    return in_maps


def kernel(**inputs):
    if "nc" not in _CACHE:
        _CACHE["nc"] = build_nc()
    nc = _CACHE["nc"]
    in_maps = host_inputs(
        d1=inputs["d1"], d2=inputs["d2"], w1=inputs["w1"], b1=inputs["b1"],
        W=inputs["W"], P=inputs["P"], wv=inputs["wv"], wcc=inputs["wcc"],
        bcc=inputs["bcc"])
    from concourse.bass_utils import run_bass_kernel_spmd
    res = run_bass_kernel_spmd(nc, in_maps, core_ids=list(range(NCORES)))
    return np.concatenate([res.results[i]["out"] for i in range(NCORES)],
                          axis=0)


# revision 22
# speedup vs baseline: 1.1820x; 1.0312x over previous
"""Trainium2 Bass kernel for nn_Attention_63513976373985.

Strategy: pure data-parallel over the batch dim B=64 across 8 NeuronCores
(8 batches per core, all params replicated, no collectives).

v3: software-pipelined schedule.
  - d2 transposed on the HOST, loaded with plain DMAs on 2 queues (no
    xbar DMA-transposes, PE starts ~2us in).
  - P-reduction (atts = P . us) is 4-way column-tiled: pat matmuls are
    emitted as 4-MM quads (heads 2k,2k+1 x ct) delayed two heads behind
    the zs/tanh producer so all four col-groups run concurrently on the
    PE with their inputs already in SBUF; a full-width [128,8] "comb"
    matmul folds the groups into atts [8,S]. Each group chain carries its
    own start/stop (the has_written clear of start=True covers only the
    addressed partition rows -- HW-verified).
  - batch b-1's softmax/vs tail is interleaved into batch b's head loop
    (exp under h=1, score-transposes under h=2, vs matmuls under h=3,
    V-slab transposes under h=4) so the PE never idles through the
    softmax latency chain and HAM stays at K=8/8.
  - d3 relu+bias on DVE (dual-op tensor_scalar); tv tanh one 1024-wide
    ACTIVATE; exp with accum_out; 1/Z folded into the vs eviction.

Everything fp16 (fp32 PSUM accumulate).
"""
import sys

if "/opt/trn_rl_repo" not in sys.path:
    sys.path.insert(0, "/opt/trn_rl_repo")

import numpy as np

H, F, C, S, B = 8, 512, 256, 512, 64
NCORES = 8
BLOC = B // NCORES  # 8
OUTF = 128

_CACHE = {}


def build_nc(debug=False, dbg_b=0):
    import concourse.bass as bass  # noqa: F401
    import concourse.mybir as mybir
    import concourse.tile as tile
    from concourse import bacc
    from contextlib import ExitStack

    f32 = mybir.dt.float32
    f16 = mybir.dt.float16
    AF = mybir.ActivationFunctionType
    ALU = mybir.AluOpType

    nc = bacc.Bacc("TRN2", target_bir_lowering=False, debug=False,
                   num_devices=NCORES)

    # ---- DRAM parameters (per-core shard shapes) ----
    xt_d = nc.dram_tensor("xt", [128, 4, BLOC, S], f16, kind="ExternalInput")
    w1_d = nc.dram_tensor("w1r", [128, 4, 2, 128], f16, kind="ExternalInput")
    wv_d = nc.dram_tensor("wvr", [128, 4, C], f16, kind="ExternalInput")
    wtop_d = nc.dram_tensor("wtopr", [128, H, 2, 2, 128], f16,
                            kind="ExternalInput")
    wbot_d = nc.dram_tensor("wbotr", [128, H, 2, 2, 128], f16,
                            kind="ExternalInput")
    wcc_d = nc.dram_tensor("wccr", [128, 2 * H, OUTF], f16,
                           kind="ExternalInput")
    # packed small consts: [0:8]=comb, [8:72]=pblk, [72:104]=d1t,
    # [104:112]=id8(rows 0:8), [112:120]=ones18(row 0), [120:248]=bcc(row 0)
    pk_d = nc.dram_tensor("packed", [128, 248], f16, kind="ExternalInput")
    b1c_d = nc.dram_tensor("b1c", [128, 2], f32, kind="ExternalInput")
    out_d = nc.dram_tensor("out", [BLOC, OUTF], f32, kind="ExternalOutput")
    if debug:
        dbg_d3t = nc.dram_tensor("dbg_d3t", [128, 2, S], f16,
                                 kind="ExternalOutput")
        dbg_tv = nc.dram_tensor("dbg_tv", [128, 4, C], f16,
                                kind="ExternalOutput")
        dbg_attsg = nc.dram_tensor("dbg_attsg", [128, S], f16,
                                   kind="ExternalOutput")
        dbg_esc = nc.dram_tensor("dbg_esc", [8, S], f16,
                                 kind="ExternalOutput")
        dbg_zc = nc.dram_tensor("dbg_zc", [128, 2, H, BLOC], f32,
                                kind="ExternalOutput")
        dbg_vs = nc.dram_tensor("dbg_vs", [BLOC, 8, C], f16,
                                kind="ExternalOutput")

    with tile.TileContext(nc) as tc, ExitStack() as stk:
        const = stk.enter_context(tc.tile_pool(name="const", bufs=1))
        xtp = stk.enter_context(tc.tile_pool(name="xtp", bufs=1))
        d3p = stk.enter_context(tc.tile_pool(name="d3p", bufs=2))
        tvpool = stk.enter_context(tc.tile_pool(name="tvpool", bufs=2))
        usp = stk.enter_context(tc.tile_pool(name="usp", bufs=4))
        smallsb = stk.enter_context(tc.tile_pool(name="smallsb", bufs=2))
        vpool = stk.enter_context(tc.tile_pool(name="vpool", bufs=1))
        pmm = stk.enter_context(tc.tile_pool(name="pmm", bufs=1, space="PSUM"))
        pzs = stk.enter_context(tc.tile_pool(name="pzs", bufs=2, space="PSUM"))
        patp = stk.enter_context(tc.tile_pool(name="patp", bufs=1,
                                              space="PSUM"))
        psmall = stk.enter_context(
            tc.tile_pool(name="psmall", bufs=1, space="PSUM"))

        # ---- DMA issue: 2 HWDGE queues (SP + Activation), few big DMAs.
        # (gpsimd.dma_start is SWDGE -- software descriptor generation with
        # ~8us startup and low throughput; avoid for everything.)
        xt_sb = xtp.tile([128, 4, BLOC, S], f16, tag="xt")
        w1_sb = const.tile([128, 4, 2, 128], f16, tag="w1")
        pk_sb = const.tile([128, 248], f16, tag="packed")
        b1c_sb = const.tile([128, 2], f32, tag="b1c")
        wv_sb = const.tile([128, 4, C], f16, tag="wv")
        wbot_sb = const.tile([128, H, 2, 2, 128], f16, tag="wbot")
        wtop_sb = const.tile([128, H, 2, 2, 128], f16, tag="wtop")
        wcc_sb = const.tile([128, 2 * H, OUTF], f16, tag="wcc")

        comb_sb = pk_sb[:, 0:8]

        def pblk_sl(idx):  # pblk[:, idx, :] from packed cols [8:72]
            return pk_sb[:, 8 + idx * 4:8 + idx * 4 + 4]

        def d1t_sl(k):  # d1t[:, k, :] from packed cols [72:104]
            return pk_sb[:, 72 + k * BLOC:72 + (k + 1) * BLOC]

        id8_sb = pk_sb[0:8, 104:112]
        ones18_sb = pk_sb[0:1, 112:120]
        bcc_sb = pk_sb[0:1, 120:248]

        # sync: w1 -> xt[b0] -> packed -> b1c -> xt[b1:4] -> xt[b4:] -> wcc
        nc.sync.dma_start(out=w1_sb, in_=w1_d[:, :, :, :])
        nc.sync.dma_start(out=xt_sb[:, :, 0, :], in_=xt_d[:, :, 0, :])
        nc.sync.dma_start(out=pk_sb, in_=pk_d[:, :])
        nc.sync.dma_start(out=b1c_sb, in_=b1c_d[:, :])
        nc.sync.dma_start(out=xt_sb[:, :, 1:4, :], in_=xt_d[:, :, 1:4, :])
        nc.sync.dma_start(out=xt_sb[:, :, 4:8, :], in_=xt_d[:, :, 4:8, :])
        nc.sync.dma_start(out=wcc_sb, in_=wcc_d[:, :, :])
        # scalar (2nd HWDGE queue): wv -> wbot -> wtop
        nc.scalar.dma_start(out=wv_sb, in_=wv_d[:, :, :])
        nc.scalar.dma_start(out=wbot_sb, in_=wbot_d[:, :, :, :, :])
        nc.scalar.dma_start(out=wtop_sb, in_=wtop_d[:, :, :, :, :])

        d4t_sb = const.tile([128, 2, BLOC], f16, tag="d4t")

        def emit_d4():
            pd4 = psmall.tile([128, 2, BLOC], f32, tag="small")
            for m in range(2):
                for k in range(4):
                    nc.tensor.matmul(pd4[:, m, :], lhsT=w1_sb[:, k, m, :],
                                     rhs=d1t_sl(k),
                                     start=(k == 0), stop=(k == 3))
            for m in range(2):
                nc.scalar.activation(d4t_sb[:, m, :], pd4[:, m, :], AF.Relu,
                                     bias=b1c_sb[:, m:m + 1])

        # ---- atts accumulator bank: zero once (garbage rows stay 0) ----
        pat = patp.tile([128, S], f32, tag="atts")
        nc.vector.memset(pat[:, :], 0.0)

        v_sb = vpool.tile([128, 2, H, BLOC], f16)  # [c-in-half, ch, h, b]
        zc_sb = const.tile([128, 2, H, BLOC], f32, tag="zc")

        # ---- pipeline stage emitters ----
        d3ts = [None] * BLOC
        tvs = [None] * BLOC
        uss = {}
        pat2s = [None] * BLOC
        nmaxs = [None] * BLOC
        escs = [None] * BLOC
        zinvs = [None] * BLOC
        scts = [None] * BLOC
        vssbs = [None] * BLOC

        def emit_d3_tv(b):
            xt = xt_sb[:, :, b, :]
            pmd3 = pmm.tile([128, 2, S], f32, tag="mm", name=f"pmd3_{b}")
            for m in range(2):
                for kf in range(4):
                    nc.tensor.matmul(pmd3[:, m, :], lhsT=w1_sb[:, kf, m, :],
                                     rhs=xt[:, kf, :],
                                     start=(kf == 0), stop=(kf == 3))
            d3t = d3p.tile([128, 2, S], f16, tag="d3t", name=f"d3t{b}")
            for m in range(2):
                nc.vector.tensor_scalar(
                    d3t[:, m, :], pmd3[:, m, :],
                    scalar1=b1c_sb[:, m:m + 1], scalar2=0.0,
                    op0=ALU.add, op1=ALU.max)
            d3ts[b] = d3t
            pmtv = pmm.tile([128, 4, C], f32, tag="mm", name=f"pmtv_{b}")
            for sc in range(4):
                for kf in range(4):
                    nc.tensor.matmul(
                        pmtv[:, sc, :],
                        lhsT=xt[:, kf, sc * 128:(sc + 1) * 128],
                        rhs=wv_sb[:, kf, :],
                        start=(kf == 0), stop=(kf == 3))
            tv = tvpool.tile([128, 4, C], f16, tag="tv", name=f"tv{b}")
            nc.scalar.activation(tv[:, :, :], pmtv[:, :, :], AF.Tanh)
            tvs[b] = tv
            if debug and b == dbg_b:
                nc.sync.dma_start(out=dbg_d3t[:, :, :], in_=d3t)
                nc.sync.dma_start(out=dbg_tv[:, :, :], in_=tv)

        def emit_zc():
            pzc = psmall.tile([128, 2, H, BLOC], f32, tag="small")
            for ct in range(2):
                for h in range(H):
                    for ks in range(2):
                        nc.tensor.matmul(pzc[:, ct, h, :],
                                         lhsT=wbot_sb[:, h, ks, ct, :],
                                         rhs=d4t_sb[:, ks, :],
                                         start=(ks == 0), stop=(ks == 1))
            nc.vector.tensor_copy(out=zc_sb, in_=pzc)
            if debug:
                nc.sync.dma_start(out=dbg_zc[:, :, :, :], in_=zc_sb)

        def emit_zs(b, h):
            pz = pzs.tile([128, 2, S], f32, tag="zs", name=f"pz{b}_{h}")
            for ct in range(2):
                for ks in range(2):
                    nc.tensor.matmul(pz[:, ct, :],
                                     lhsT=wtop_sb[:, h, ks, ct, :],
                                     rhs=d3ts[b][:, ks, :],
                                     start=(ks == 0), stop=(ks == 1))
            us = usp.tile([128, 2, S], f16, tag="us", name=f"us{b}_{h}")
            for ct in range(2):
                nc.scalar.activation(us[:, ct, :], pz[:, ct, :], AF.Tanh,
                                     bias=zc_sb[:, ct, h, b:b + 1])
            uss[(b, h)] = us

        def emit_pat_quad(b, k):
            # 4 adjacent matmuls covering all 4 col-groups -> concurrent.
            for hh in (2 * k, 2 * k + 1):
                us = uss.pop((b, hh))
                for ct in range(2):
                    g = 2 * (hh % 2) + ct
                    nc.tensor.matmul(pat[32 * g:32 * g + 4, :],
                                     lhsT=pblk_sl(hh * 2 + ct),
                                     rhs=us[:, ct, :],
                                     start=(hh < 2), stop=(hh >= 6),
                                     skip_group_check=True,
                                     tile_position=(0, 32 * g))

        def emit_softmax_pre(b):
            attsg = smallsb.tile([128, S], f16, tag="attsg", name=f"ag{b}")
            nc.vector.tensor_copy(out=attsg, in_=pat)
            if debug and b == dbg_b:
                nc.sync.dma_start(out=dbg_attsg[:, :], in_=attsg)
            pat2 = psmall.tile([8, S], f32, tag="small", name=f"pat2_{b}")
            nc.tensor.matmul(pat2, lhsT=comb_sb, rhs=attsg,
                             start=True, stop=True)
            nmax = smallsb.tile([8, 1], f32, tag="nmax", name=f"nmax{b}")
            nc.vector.tensor_reduce(nmax, pat2, axis=mybir.AxisListType.X,
                                    op=ALU.max, negate=True)
            pat2s[b] = pat2
            nmaxs[b] = nmax

        def emit_softmax_exp(b):
            esc = smallsb.tile([8, S], f16, tag="esc", name=f"esc{b}")
            zsum = smallsb.tile([8, 1], f32, tag="zsum", name=f"zsum{b}")
            nc.scalar.activation(esc, pat2s[b], AF.Exp, bias=nmaxs[b],
                                 accum_out=zsum)
            zinv = smallsb.tile([8, 1], f32, tag="zinv", name=f"zinv{b}")
            nc.vector.reciprocal(zinv, zsum)
            escs[b] = esc
            zinvs[b] = zinv
            if debug and b == dbg_b:
                nc.sync.dma_start(out=dbg_esc[:, :], in_=esc)

        def emit_tail_psc(b):
            psc = psmall.tile([128, 4, 8], f16, tag="small", name=f"psc{b}")
            for sc in range(4):
                nc.tensor.transpose(psc[:, sc, :],
                                    in_=escs[b][:, sc * 128:(sc + 1) * 128],
                                    identity=id8_sb)
            sct = smallsb.tile([128, 4, 8], f16, tag="sct", name=f"sct{b}")
            nc.vector.tensor_copy(out=sct, in_=psc)
            scts[b] = sct

        def emit_tail_vs(b):
            pvs = psmall.tile([8, C], f32, tag="small", name=f"pvs{b}")
            for sc in range(4):
                nc.tensor.matmul(pvs, lhsT=scts[b][:, sc, :],
                                 rhs=tvs[b][:, sc, :],
                                 start=(sc == 0), stop=(sc == 3))
            vssb = smallsb.tile([8, C], f16, tag="vssb", name=f"vssb{b}")
            nc.vector.tensor_scalar_mul(vssb, pvs, zinvs[b])
            vssbs[b] = vssb
            if debug:
                nc.sync.dma_start(out=dbg_vs[b, :, :], in_=vssb)

        def emit_tail_pvt(b):
            pvt = psmall.tile([128, 2, 8], f16, tag="small", name=f"pvt{b}")
            for ch in range(2):
                nc.tensor.transpose(
                    pvt[:, ch, :],
                    in_=vssbs[b][:, ch * 128:(ch + 1) * 128],
                    identity=id8_sb)
            for ch in range(2):
                nc.vector.tensor_copy(out=v_sb[:, ch, :, b:b + 1],
                                      in_=pvt[:, ch, :])

        # ---- pipelined emission ----
        emit_d3_tv(0)
        emit_d4()
        emit_zc()
        for b in range(BLOC):
            for h in range(H):
                emit_zs(b, h)
                if h == 2:
                    emit_pat_quad(b, 0)
                if h == 4:
                    emit_pat_quad(b, 1)
                if h == 6:
                    emit_pat_quad(b, 2)
                if b > 0:
                    if h == 0:
                        emit_softmax_pre(b - 1)
                    elif h == 1:
                        emit_softmax_exp(b - 1)
                    elif h == 2:
                        emit_tail_psc(b - 1)
                    elif h == 3:
                        emit_tail_vs(b - 1)
                    elif h == 4:
                        emit_tail_pvt(b - 1)
            if b + 1 < BLOC:
                emit_d3_tv(b + 1)
            emit_pat_quad(b, 3)
        emit_softmax_pre(BLOC - 1)
        emit_softmax_exp(BLOC - 1)
        emit_tail_psc(BLOC - 1)
        emit_tail_vs(BLOC - 1)
        emit_tail_pvt(BLOC - 1)

        # ---- final: out = relu(V.T @ wcc + bcc) ----
        pout = psmall.tile([8, OUTF], f32, tag="small")
        kidx = 0
        for h in range(H):
            for ch in range(2):
                nc.tensor.matmul(pout, lhsT=v_sb[:, ch, h, :],
                                 rhs=wcc_sb[:, h * 2 + ch, :],
                                 start=(kidx == 0), stop=False)
                kidx += 1
        nc.tensor.matmul(pout, lhsT=ones18_sb, rhs=bcc_sb,
                         start=False, stop=True)
        outsb = smallsb.tile([8, OUTF], f32, tag="outsb")
        nc.scalar.activation(outsb, pout, AF.Relu)
        nc.sync.dma_start(out=out_d[:, :], in_=outsb)

    nc.compile()
    return nc


def host_inputs(d1, d2, w1, b1, W, P, wv, wcc, bcc):
    """Host-side sharding + layout prep. Returns in_maps for 8 cores."""
    d1 = np.ascontiguousarray(d1, dtype=np.float32)
    d2 = np.ascontiguousarray(d2, dtype=np.float32)
    w1 = np.ascontiguousarray(w1, dtype=np.float32)
    b1 = np.ascontiguousarray(b1, dtype=np.float32)
    W = np.ascontiguousarray(W, dtype=np.float32)
    P = np.ascontiguousarray(P, dtype=np.float32)
    wv = np.ascontiguousarray(wv, dtype=np.float32)
    wcc = np.ascontiguousarray(wcc, dtype=np.float32)
    bcc = np.ascontiguousarray(bcc, dtype=np.float32)

    w1r = np.ascontiguousarray(
        w1.reshape(4, 128, 2, 128).transpose(1, 0, 2, 3))
    wvr = np.ascontiguousarray(wv.reshape(4, 128, C).transpose(1, 0, 2))
    wtopr = np.ascontiguousarray(
        W[:, :C, :].reshape(H, 2, 128, 2, 128).transpose(2, 0, 1, 3, 4))
    wbotr = np.ascontiguousarray(
        W[:, C:, :].reshape(H, 2, 128, 2, 128).transpose(2, 0, 1, 3, 4))
    # 4-way col-tiled P blocks: head h, half ct -> col-group g=2*(h%2)+ct,
    # output row r=h//2 within the group.
    pblkr = np.zeros((128, 2 * H, 4), np.float32)
    combr = np.zeros((128, H), np.float32)
    for h in range(H):
        r = h // 2
        for ct in range(2):
            g = 2 * (h % 2) + ct
            pblkr[:, h * 2 + ct, r] = P[h, ct * 128:(ct + 1) * 128]
            combr[32 * g + r, h] = 1.0
    wccr = np.ascontiguousarray(
        wcc.reshape(2 * H, 128, OUTF).transpose(1, 0, 2))
    bccr = np.ascontiguousarray(bcc[None, :])
    b1c = np.ascontiguousarray(b1.reshape(2, 128).T)
    id8 = np.eye(8, dtype=np.float32)
    ones18 = np.ones((1, 8), np.float32)

    f16 = np.float16
    packed0 = np.zeros((128, 248), np.float32)
    packed0[:, 0:8] = combr
    packed0[:, 8:72] = pblkr.reshape(128, 64)
    packed0[0:8, 104:112] = id8
    packed0[0:1, 112:120] = ones18
    packed0[0:1, 120:248] = bccr
    shared = dict(w1r=w1r.astype(f16), wvr=wvr.astype(f16),
                  wtopr=wtopr.astype(f16), wbotr=wbotr.astype(f16),
                  wccr=wccr.astype(f16), b1c=b1c)
    in_maps = []
    for core in range(NCORES):
        bs = slice(core * BLOC, (core + 1) * BLOC)
        # xt[p, kf, b, s] = d2[s, bs.start+b, kf*128+p]
        d2c = d2[:, bs, :]  # [S, BLOC, F]
        xtr = np.ascontiguousarray(
            d2c.transpose(2, 1, 0).reshape(4, 128, BLOC, S)
            .transpose(1, 0, 2, 3)).astype(np.float16)
        d1c = d1[bs]  # [BLOC, F]
        d1tr = d1c.T.reshape(4, 128, BLOC).transpose(1, 0, 2)
        packed = packed0.copy()
        packed[:, 72:104] = d1tr.reshape(128, 32)
        in_maps.append(dict(xt=xtr, packed=packed.astype(f16), **shared))
    return in_maps


def kernel(**inputs):
    if "nc" not in _CACHE:
        _CACHE["nc"] = build_nc()
    nc = _CACHE["nc"]
    in_maps = host_inputs(
        d1=inputs["d1"], d2=inputs["d2"], w1=inputs["w1"], b1=inputs["b1"],
        W=inputs["W"], P=inputs["P"], wv=inputs["wv"], wcc=inputs["wcc"],
        bcc=inputs["bcc"])
    from concourse.bass_utils import run_bass_kernel_spmd
    res = run_bass_kernel_spmd(nc, in_maps, core_ids=list(range(NCORES)))
    return np.concatenate([res.results[i]["out"] for i in range(NCORES)],
                          axis=0)
